# revision 9
# baseline (speedup 1.0000x reference)
"""Trainium2 Bass kernel for nn_CELoss_4896262717859.

For each query column c = idx_node[k] of a sparse adjacency matrix (diagonal
zeroed), a cross-entropy-style loss over the "lower" (r < c) and "upper"
(r >= c) neighbor sets:

    contrib_side(c) = [cnt>0 and poscnt==1] * (lse - poslogit) / cnt

Strategy (v2):
  * Host gathers ONLY the K=4096 needed columns G = node_adj[:, idx_node]
    (diagonal zeroed) -> fp8 (values 0/1, exact), halving device traffic vs
    processing all N columns, and quartering bytes vs int32.
  * Device: per 512-column core slab, 32 fp8 DoubleRow matmuls — each
    covers a PAIR of 128-row tiles at 0.5 PE cycles/column — producing
    per-pair stats [6, 512] = {ones, pos, pl_hi, pl_lo*SC, e_hi, e_lo*SC}
    sums. No masking, no dtype casts, fully static shapes.
  * Host combine: the L/U split boundary (row idx_node[k]) is handled by a
    prefix sum over the 32 per-pair stats plus an exact 256-row partial for
    the boundary pair, then the scalar CE reduction. O(K) work.

Sharding: columns split into 8 slabs of 512 (one per core); every core runs
the identical NEFF (same weights), only the G slab differs.
"""

import numpy as np
import ml_dtypes

N = 8192
K = 4096
NCORES = 8
SLAB = K // NCORES        # 512 columns per core
P = 128                   # partition / tile edge
NT = N // P               # 64 row tiles
PAIRS = NT // 2           # 32 row-tile pairs (DoubleRow granularity)
ROWS_PER_PAIR = 2 * P     # 256
NW = 6                    # stat components per column
SC = 32.0                 # scale for the *_lo fp8 channels
NWP = 16                  # weight inner-dim padding: dual-fp8 ldweights
                          # requires a 16-byte-aligned k-plane stride
CHUNK = 4                 # row tiles per input DMA
NCH = NT // CHUNK         # 16 input DMAs

FP8 = ml_dtypes.float8_e4m3   # == mybir.dt.np(mybir.dt.float8e4); max 240

_BASS_CACHE = {}


def _build_bass():
    import concourse.tile as tile
    import concourse.mybir as mybir
    from concourse import bacc

    nc = bacc.Bacc("TRN2")
    # g[c, p, u, n] = adjacency row 128*(CHUNK*c+u)+p, slab column n (0/1 fp8).
    # Chunk-major so each chunk DMA reads one fully contiguous DRAM block
    # (strided 4KB reads at 32KB pitch only reach ~260GB/s vs ~356 contiguous).
    g = nc.dram_tensor(
        "g", [NCH, P, CHUNK, SLAB], mybir.dt.float8e4, kind="ExternalInput"
    )
    # w[p, j, m] = weight component m for row 128*j+p
    w = nc.dram_tensor("w", [P, NT, NWP], mybir.dt.float8e4, kind="ExternalInput")
    stats = nc.dram_tensor(
        "stats", [NW, PAIRS * SLAB], mybir.dt.float32, kind="ExternalOutput"
    )

    NWARM = 9
    with tile.TileContext(nc) as tc:
        with (
            tc.tile_pool(name="singles", bufs=1) as singles,
            tc.tile_pool(name="io", bufs=NCH) as io_pool,
            tc.tile_pool(name="psum", bufs=7, space="PSUM") as psum_pool,
            tc.tile_pool(name="warm", bufs=1, space="PSUM") as warm_pool,
        ):
            # weights on the ACT HW queue: sync's queue stays a pure,
            # in-order, full-bandwidth g stream
            wsb = singles.tile([P, NT, NWP], mybir.dt.float8e4)
            nc.scalar.dma_start(out=wsb, in_=w[:, :, :])
            chunks = []
            for c in range(NCH):
                t = io_pool.tile(
                    [P, CHUNK, SLAB], mybir.dt.float8e4, tag="g", name=f"g{c}"
                )
                nc.sync.dma_start(out=t, in_=g[c, :, :, :])
                chunks.append(t)

            # PE p-state warm-up: the tensor engine only reaches full clock
            # after ~3us of continuous execution. Zero-matmul on a scratch
            # tile (no input deps) from the end of the preamble until real
            # data lands, so the real matmuls run at ~216ns, not ~427ns.
            scratch = singles.tile([P, 2, SLAB], mybir.dt.float8e4)
            nc.gpsimd.memset(scratch, 0)
            wacc = warm_pool.tile([NW, SLAB], mybir.dt.float32, name="wacc")
            for i in range(NWARM):
                nc.tensor.matmul(
                    wacc,
                    scratch[:, :, 0:NW],
                    scratch[:, :, :],
                    start=True,
                    stop=True,
                    perf_mode=mybir.MatmulPerfMode.DoubleRow,
                )

            out_sb = singles.tile([NW, PAIRS * SLAB], mybir.dt.float32)

            for q in range(PAIRS):
                acc = psum_pool.tile(
                    [NW, SLAB], mybir.dt.float32, tag="acc", name=f"acc{q}"
                )
                ch = chunks[(2 * q) // CHUNK]
                off = (2 * q) % CHUNK
                # DoubleRow: lhsT [128, 2, 6], rhs [128, 2, 512] -> out [6, 512]
                # = sum over the two 128-row tiles at 0.5 cycles/column.
                nc.tensor.matmul(
                    acc,
                    wsb[:, 2 * q : 2 * q + 2, 0:NW],
                    ch[:, off : off + 2, :],
                    start=True,
                    stop=True,
                    perf_mode=mybir.MatmulPerfMode.DoubleRow,
                )
                # psum -> sbuf staging alternates DVE/ACT so neither engine's
                # ~570ns copy chain paces the matmul stream
                dst = out_sb[:, q * SLAB : (q + 1) * SLAB]
                if q % 2 == 0:
                    nc.vector.tensor_copy(dst, acc)
                else:
                    nc.scalar.copy(dst, acc)
                # stats out on sync's queue, after its g issues are done;
                # two halves so the first mostly hides under the stream
                if (q + 1) % (PAIRS // 2) == 0:
                    s = (q + 1 - PAIRS // 2) * SLAB
                    e = (q + 1) * SLAB
                    nc.sync.dma_start(out=stats[:, s:e], in_=out_sb[:, s:e])

    nc.compile()
    return nc


def _host_prep(outputs, targets):
    """Quantized weight table [8192, 6] fp8 + exact f64 weights [8192, 4]."""
    out = np.asarray(outputs, np.float64).reshape(-1)
    pos = (np.asarray(targets).reshape(-1) != 0).astype(np.float64)
    # shift exp into fp8 range only if needed (max normal 240 -> ln 240 = 5.48)
    b_shift = max(0.0, float(out.max()) - 4.5)
    pl = pos * out
    ev = np.exp(out - b_shift)

    def split(v):
        hi = v.astype(FP8)
        lo = ((v - hi.astype(np.float64)) * SC).astype(FP8)
        return hi, lo

    pl_hi, pl_lo = split(pl)
    e_hi, e_lo = split(ev)
    wq = np.stack(
        [
            np.ones(N, FP8),
            pos.astype(FP8),
            pl_hi,
            pl_lo,
            e_hi,
            e_lo,
        ],
        axis=1,
    )  # [N, 6] fp8
    wtrue = np.stack([np.ones(N), pos, pl, ev], axis=1)  # [N, 4] f64
    wpad = np.zeros((N, NWP), FP8)
    wpad[:, :NW] = wq
    wmat = np.ascontiguousarray(wpad.reshape(NT, P, NWP).transpose(1, 0, 2))
    return wmat, wtrue, b_shift


def _gather_columns(node_adj, idx_node):
    """G[r, k] = node_adj[r, idx[k]] != 0, diag zeroed. uint8 [N, K]."""
    idx = np.asarray(idx_node).reshape(-1).astype(np.int64)
    G = (np.asarray(node_adj)[:, idx] != 0).astype(np.uint8)
    G[idx, np.arange(K)] = 0  # node_adj[diag] = 0
    return G, idx


def _build_shard(G, core):
    """Per-core [NCH, 128, CHUNK, 512] fp8 from column slab [N, 512]."""
    cols = G[:, core * SLAB : (core + 1) * SLAB]
    arr = cols.reshape(NCH, CHUNK, P, SLAB).transpose(0, 2, 1, 3)
    return np.ascontiguousarray(arr).astype(FP8)


def _combine(stats_list, idx, G, wtrue, b_shift):
    """stats_list: per-core [6, PAIRS*512] f32 -> scalar loss (f64 math)."""
    # Sg[q, m, k]: per-pair stats for all K columns
    Sg = np.empty((PAIRS, NW, K), np.float64)
    for c, s in enumerate(stats_list):
        Sg[:, :, c * SLAB : (c + 1) * SLAB] = (
            np.asarray(s, np.float64).reshape(NW, PAIRS, SLAB).transpose(1, 0, 2)
        )
    C = np.concatenate(
        [np.zeros((1, NW, K)), np.cumsum(Sg, axis=0)], axis=0
    )  # [PAIRS+1, 6, K]

    kk = np.arange(K)
    qk = (idx // ROWS_PER_PAIR).astype(np.int64)
    L_raw = C[qk, :, kk]                        # [K, 6] full pairs below boundary
    U_raw = C[PAIRS, :, kk] - C[qk + 1, :, kk]  # full pairs above boundary

    def unpack(raw):
        cnt = raw[:, 0]
        poscnt = raw[:, 1]
        pl = raw[:, 2] + raw[:, 3] / SC
        ev = raw[:, 4] + raw[:, 5] / SC
        return cnt, poscnt, pl, ev

    # exact f64 partial for the boundary pair (256 rows containing idx[k])
    rows = qk[None, :] * ROWS_PER_PAIR + np.arange(ROWS_PER_PAIR)[:, None]  # [256,K]
    gpair = G[rows, kk[None, :]].astype(np.float64)
    low = (rows < idx[None, :]).astype(np.float64)
    glo = gpair * low
    ghi = gpair - glo

    def partial(gm):
        return [
            gm.sum(axis=0),
            (gm * wtrue[rows, 1]).sum(axis=0),
            (gm * wtrue[rows, 2]).sum(axis=0),
            (gm * wtrue[rows, 3]).sum(axis=0),
        ]

    def side(raw, gm):
        cnt, poscnt, pl, ev = unpack(raw)
        pc, pp, ppl, pe = partial(gm)
        cnt = cnt + pc
        poscnt = poscnt + pp
        pl = pl + ppl
        ev = ev + pe
        valid = (cnt > 0.5) & (np.abs(poscnt - 1.0) < 0.25)
        lse = np.log(np.where(valid, np.maximum(ev, 1e-300), 1.0)) + b_shift
        return np.where(valid, (lse - pl) / np.maximum(cnt, 1.0), 0.0).sum()

    return np.array(side(L_raw, glo) + side(U_raw, ghi), dtype=np.float32)


def _ensure_axon_hooks_stub():
    """bass_utils imports antenv.axon_hooks when tracing is requested via
    env; the module is absent on some images. Provide a no-op stub so the
    import never crashes (hook=None -> bass_utils skips tracing)."""
    import sys
    import types

    try:
        import antenv.axon_hooks  # noqa: F401
    except ImportError:
        mod = types.ModuleType("antenv.axon_hooks")
        state = {"hook": None}
        mod.set_axon_ntff_profile_hook = lambda h: state.__setitem__("hook", h)
        mod.get_axon_ntff_profile_hook = lambda: state["hook"]
        sys.modules["antenv.axon_hooks"] = mod


def _device_stats(in_maps):
    _ensure_axon_hooks_stub()
    from concourse.bass_utils import run_bass_kernel_spmd

    if "nc" not in _BASS_CACHE:
        _BASS_CACHE["nc"] = _build_bass()
    last_exc = None
    for attempt in range(4):
        try:
            res = run_bass_kernel_spmd(
                _BASS_CACHE["nc"], in_maps, core_ids=list(range(NCORES))
            )
            return [r["stats"] for r in res.results]
        except Exception as e:  # transient NRT/accelerator hiccups
            last_exc = e
            try:
                # a fresh PJRT client usually recovers a transiently
                # "unrecoverable" accelerator; mirrors a process restart
                import jax
                import jax.extend.backend as _jeb

                jax.clear_caches()
                _jeb.clear_backends()
            except Exception:
                pass
            import time

            time.sleep(2.0 * (attempt + 1))
    raise last_exc


def _sim_stats(in_maps):
    """Numpy emulation of the device kernel (same inputs), for logic validation."""
    outs = []
    for m in in_maps:
        # [NCH, 128, CHUNK, 512] -> [128, 64, 512]
        gm = m["g"].astype(np.float32).transpose(1, 0, 2, 3).reshape(P, NT, SLAB)
        wm = m["w"].astype(np.float32)[:, :, :NW]  # [128, 64, 6]
        acc = np.zeros((NW, PAIRS, SLAB), np.float32)
        for q in range(PAIRS):
            for j in (2 * q, 2 * q + 1):
                acc[:, q, :] += wm[:, j, :].T @ gm[:, j, :]
        outs.append(acc.reshape(NW, PAIRS * SLAB))
    return outs


def _prep(outputs, targets, node_adj, idx_node):
    wmat, wtrue, b_shift = _host_prep(outputs, targets)
    G, idx = _gather_columns(node_adj, idx_node)
    in_maps = [{"g": _build_shard(G, d), "w": wmat} for d in range(NCORES)]
    return in_maps, (idx, G, wtrue, b_shift)


def kernel(outputs, targets, node_adj, idx_node, _simulate=False):
    in_maps, ctx = _prep(outputs, targets, node_adj, idx_node)
    stats = _sim_stats(in_maps) if _simulate else _device_stats(in_maps)
    return _combine(stats, *ctx)


# revision 10
# speedup vs baseline: 1.1676x; 1.1676x over previous
"""Trainium2 Bass kernel for nn_CELoss_4896262717859.

For each query column c = idx_node[k] of a sparse adjacency matrix (diagonal
zeroed), a cross-entropy-style loss over the "lower" (r < c) and "upper"
(r >= c) neighbor sets:

    contrib_side(c) = [cnt>0 and poscnt==1] * (lse - poslogit) / cnt

Strategy (v5):
  * Host gathers only the UNIQUE query columns (~3.2K of 8192, idx_node has
    ~21% duplicates) -> fp8 (values 0/1, exact). Duplicates are applied as
    multiplicities in the host combine. ~3.4MB/core vs 33.5MB int32 baseline.
  * Device: per-core column slab, 32 fp8 DoubleRow matmuls — each covers a
    PAIR of 128-row tiles at 0.5 PE cycles/column — producing per-pair stats
    [6, slab] = {ones, pos, pl_hi, pl_lo*SC, e_hi, e_lo*SC}. No masking, no
    casts; psum->sbuf copies alternate DVE/ACT; one in-order full-bandwidth
    DMA stream on the sync queue.
  * Host combine: the L/U split at row idx_node[k] = prefix sum over the 32
    per-pair stats + an exact 256-row partial for the boundary pair. O(K).
"""

import numpy as np
import ml_dtypes

N = 8192
K = 4096
NCORES = 8
P = 128                   # partition / tile edge
NT = N // P               # 64 row tiles
PAIRS = NT // 2           # 32 row-tile pairs (DoubleRow granularity)
ROWS_PER_PAIR = 2 * P     # 256
NW = 6                    # stat components per column
SC = 32.0                 # scale for the *_lo fp8 channels
NWP = 16                  # weight inner-dim padding: dual-fp8 ldweights
                          # requires a 16-byte-aligned k-plane stride
CHUNK = 8                 # row tiles per input DMA
NCH = NT // CHUNK         # 8 input DMAs

FP8 = ml_dtypes.float8_e4m3   # == mybir.dt.np(mybir.dt.float8e4); max 240

_BASS_CACHE = {}


def _build_bass(slab):
    import concourse.tile as tile
    import concourse.mybir as mybir
    from concourse import bacc

    nc = bacc.Bacc("TRN2")
    # g[c, p, u, n] = adjacency row 128*(CHUNK*c+u)+p, slab column n (0/1 fp8).
    # Chunk-major so each chunk DMA reads one fully contiguous DRAM block
    # (strided 4KB reads at 32KB pitch only reach ~260GB/s vs ~356 contiguous).
    g = nc.dram_tensor(
        "g", [NCH, P, CHUNK, slab], mybir.dt.float8e4, kind="ExternalInput"
    )
    # w[p, j, m] = weight component m for row 128*j+p
    w = nc.dram_tensor("w", [P, NT, NWP], mybir.dt.float8e4, kind="ExternalInput")
    stats = nc.dram_tensor(
        "stats", [NW, PAIRS * slab], mybir.dt.float32, kind="ExternalOutput"
    )

    with tile.TileContext(nc) as tc:
        with (
            tc.tile_pool(name="singles", bufs=1) as singles,
            tc.tile_pool(name="io", bufs=NCH) as io_pool,
            tc.tile_pool(name="psum", bufs=8, space="PSUM") as psum_pool,
        ):
            # weights on the ACT HW queue: sync's queue stays a pure,
            # in-order, full-bandwidth g stream
            wsb = singles.tile([P, NT, NWP], mybir.dt.float8e4)
            nc.scalar.dma_start(out=wsb, in_=w[:, :, :])
            chunks = []
            for c in range(NCH):
                t = io_pool.tile(
                    [P, CHUNK, slab], mybir.dt.float8e4, tag="g", name=f"g{c}"
                )
                nc.sync.dma_start(out=t, in_=g[c, :, :, :])
                chunks.append(t)

            out_sb = singles.tile([NW, PAIRS * slab], mybir.dt.float32)

            for q in range(PAIRS):
                acc = psum_pool.tile(
                    [NW, slab], mybir.dt.float32, tag="acc", name=f"acc{q}"
                )
                ch = chunks[(2 * q) // CHUNK]
                off = (2 * q) % CHUNK
                # DoubleRow: lhsT [128, 2, 6], rhs [128, 2, slab] -> [6, slab]
                # = sum over the two 128-row tiles at 0.5 cycles/column.
                nc.tensor.matmul(
                    acc,
                    wsb[:, 2 * q : 2 * q + 2, 0:NW],
                    ch[:, off : off + 2, :],
                    start=True,
                    stop=True,
                    perf_mode=mybir.MatmulPerfMode.DoubleRow,
                )
                # psum -> sbuf staging alternates DVE/ACT so neither engine's
                # ~570ns copy chain paces the matmul stream
                dst = out_sb[:, q * slab : (q + 1) * slab]
                if q % 2 == 0:
                    nc.vector.tensor_copy(dst, acc)
                else:
                    nc.scalar.copy(dst, acc)
                # stats out in 4 groups on sync's queue (idle once the g
                # issues are done) so ACT never stalls its copy chain
                if (q + 1) % (PAIRS // 4) == 0:
                    s = (q + 1 - PAIRS // 4) * slab
                    e = (q + 1) * slab
                    nc.sync.dma_start(out=stats[:, s:e], in_=out_sb[:, s:e])

    nc.compile()
    return nc


def _host_prep(outputs, targets):
    """Quantized weight table [128, 64, 16] fp8 + exact f64 weights [8192, 4]."""
    out = np.asarray(outputs, np.float64).reshape(-1)
    pos = (np.asarray(targets).reshape(-1) != 0).astype(np.float64)
    # shift exp into fp8 range only if needed (max normal 240 -> ln 240 = 5.48)
    b_shift = max(0.0, float(out.max()) - 4.5)
    pl = pos * out
    ev = np.exp(out - b_shift)

    def split(v):
        hi = v.astype(FP8)
        lo = ((v - hi.astype(np.float64)) * SC).astype(FP8)
        return hi, lo

    pl_hi, pl_lo = split(pl)
    e_hi, e_lo = split(ev)
    wq = np.stack(
        [np.ones(N, FP8), pos.astype(FP8), pl_hi, pl_lo, e_hi, e_lo], axis=1
    )  # [N, 6] fp8
    wtrue = np.stack([np.ones(N), pos, pl, ev], axis=1)  # [N, 4] f64
    wpad = np.zeros((N, NWP), FP8)
    wpad[:, :NW] = wq
    wmat = np.ascontiguousarray(wpad.reshape(NT, P, NWP).transpose(1, 0, 2))
    return wmat, wtrue, b_shift


def _build_shard(Gpad, core, slab):
    """Per-core [NCH, 128, CHUNK, slab] fp8 from column slab [N, slab]."""
    cols = Gpad[:, core * slab : (core + 1) * slab]
    arr = cols.reshape(NCH, CHUNK, P, slab).transpose(0, 2, 1, 3)
    return np.ascontiguousarray(arr).astype(FP8)


def _prep(outputs, targets, node_adj, idx_node):
    wmat, wtrue, b_shift = _host_prep(outputs, targets)
    idx = np.asarray(idx_node).reshape(-1).astype(np.int64)
    uniq, counts = np.unique(idx, return_counts=True)
    ku = uniq.size
    slab = -(-ku // (NCORES * 16)) * 16      # per-core columns, multiple of 16
    kpad = slab * NCORES
    # G[r, k] = node_adj[r, uniq[k]] != 0, diag zeroed; zero-padded columns
    G = np.zeros((N, kpad), np.uint8)
    G[:, :ku] = np.asarray(node_adj)[:, uniq] != 0
    G[uniq, np.arange(ku)] = 0               # node_adj[diag] = 0
    in_maps = [{"g": _build_shard(G, d, slab), "w": wmat} for d in range(NCORES)]
    ctx = {
        "uniq": uniq, "counts": counts, "G": G, "wtrue": wtrue,
        "b_shift": b_shift, "slab": slab, "ku": ku,
    }
    return in_maps, ctx


def _combine(stats_list, ctx):
    """stats_list: per-core [6, PAIRS*slab] f32 -> scalar loss (f64 math)."""
    uniq, counts, G, wtrue, b_shift, slab, ku = (
        ctx["uniq"], ctx["counts"], ctx["G"], ctx["wtrue"],
        ctx["b_shift"], ctx["slab"], ctx["ku"],
    )
    kpad = slab * NCORES
    # Sg[q, m, k]: per-pair stats for all padded columns
    Sg = np.empty((PAIRS, NW, kpad), np.float64)
    for c, s in enumerate(stats_list):
        Sg[:, :, c * slab : (c + 1) * slab] = (
            np.asarray(s, np.float64).reshape(NW, PAIRS, slab).transpose(1, 0, 2)
        )
    Sg = Sg[:, :, :ku]
    C = np.concatenate(
        [np.zeros((1, NW, ku)), np.cumsum(Sg, axis=0)], axis=0
    )  # [PAIRS+1, 6, ku]

    kk = np.arange(ku)
    qk = (uniq // ROWS_PER_PAIR).astype(np.int64)
    L_raw = C[qk, :, kk]                        # [ku, 6] full pairs below split
    U_raw = C[PAIRS, :, kk] - C[qk + 1, :, kk]  # full pairs above split

    # exact f64 partial for the boundary pair (256 rows containing uniq[k])
    rows = qk[None, :] * ROWS_PER_PAIR + np.arange(ROWS_PER_PAIR)[:, None]
    gpair = G[rows, kk[None, :]].astype(np.float64)
    low = (rows < uniq[None, :]).astype(np.float64)
    glo = gpair * low
    ghi = gpair - glo

    def side(raw, gm):
        cnt = raw[:, 0] + gm.sum(axis=0)
        poscnt = raw[:, 1] + (gm * wtrue[rows, 1]).sum(axis=0)
        pl = raw[:, 2] + raw[:, 3] / SC + (gm * wtrue[rows, 2]).sum(axis=0)
        ev = raw[:, 4] + raw[:, 5] / SC + (gm * wtrue[rows, 3]).sum(axis=0)
        valid = (cnt > 0.5) & (np.abs(poscnt - 1.0) < 0.25)
        lse = np.log(np.where(valid, np.maximum(ev, 1e-300), 1.0)) + b_shift
        return np.where(valid, (lse - pl) / np.maximum(cnt, 1.0), 0.0)

    contrib = side(L_raw, glo) + side(U_raw, ghi)
    return np.array((contrib * counts).sum(), dtype=np.float32)


def _ensure_axon_hooks_stub():
    """bass_utils imports antenv.axon_hooks when tracing is requested via
    env; the module is absent on some images. Provide a no-op stub so the
    import never crashes (hook=None -> bass_utils skips tracing)."""
    import sys
    import types

    try:
        import antenv.axon_hooks  # noqa: F401
    except ImportError:
        mod = types.ModuleType("antenv.axon_hooks")
        state = {"hook": None}
        mod.set_axon_ntff_profile_hook = lambda h: state.__setitem__("hook", h)
        mod.get_axon_ntff_profile_hook = lambda: state["hook"]
        sys.modules["antenv.axon_hooks"] = mod


def _device_stats(in_maps, slab):
    _ensure_axon_hooks_stub()
    from concourse.bass_utils import run_bass_kernel_spmd

    if slab not in _BASS_CACHE:
        _BASS_CACHE[slab] = _build_bass(slab)
    last_exc = None
    for attempt in range(4):
        try:
            res = run_bass_kernel_spmd(
                _BASS_CACHE[slab], in_maps, core_ids=list(range(NCORES))
            )
            return [r["stats"] for r in res.results]
        except Exception as e:  # transient NRT/accelerator hiccups
            last_exc = e
            try:
                # a fresh PJRT client usually recovers a transiently
                # "unrecoverable" accelerator; mirrors a process restart
                import jax
                import jax.extend.backend as _jeb

                jax.clear_caches()
                _jeb.clear_backends()
            except Exception:
                pass
            import time

            time.sleep(2.0 * (attempt + 1))
    raise last_exc


def _sim_stats(in_maps, slab):
    """Numpy emulation of the device kernel (same inputs), for validation."""
    outs = []
    for m in in_maps:
        gm = m["g"].astype(np.float32).transpose(1, 0, 2, 3).reshape(P, NT, slab)
        wm = m["w"].astype(np.float32)[:, :, :NW]  # [128, 64, 6]
        acc = np.zeros((NW, PAIRS, slab), np.float32)
        for q in range(PAIRS):
            for j in (2 * q, 2 * q + 1):
                acc[:, q, :] += wm[:, j, :].T @ gm[:, j, :]
        outs.append(acc.reshape(NW, PAIRS * slab))
    return outs


def kernel(outputs, targets, node_adj, idx_node, _simulate=False):
    in_maps, ctx = _prep(outputs, targets, node_adj, idx_node)
    slab = ctx["slab"]
    stats = _sim_stats(in_maps, slab) if _simulate else _device_stats(in_maps, slab)
    return _combine(stats, ctx)
